# revision 14
# baseline (speedup 1.0000x reference)
"""AudioSNN forward pass on 8 Trainium2 NeuronCores (pure data parallel).

Host side: weight re-layout + padding (numpy). Device side: Bass/Tile kernel
per core over a 128-batch shard.
"""
import os
import sys
import numpy as np

for _p in ("/opt/trn_rl_repo", "/root/.axon_site/_ro/trn_rl_repo"):
    if os.path.isdir(_p) and _p not in sys.path:
        sys.path.insert(0, _p)

import ml_dtypes
from contextlib import ExitStack

import concourse.bass as bass
import concourse.tile as tile
from concourse import mybir, bacc
from concourse.bass_utils import run_bass_kernel_spmd

BF16 = mybir.dt.bfloat16
F32 = mybir.dt.float32
GT = mybir.AluOpType.is_gt
ADD = mybir.AluOpType.add
SUB = mybir.AluOpType.subtract
MUL = mybir.AluOpType.mult
SIGMOID = mybir.ActivationFunctionType.Sigmoid

N_CORES = 8
B = 1024
BL = B // N_CORES        # 128 batch per core
H, W = 64, 32            # conv1 image
HP, WP = H + 3, W + 3    # padded (67, 35); conv1 uses a 4x4 tap window
H2, W2 = 32, 16          # conv2 image (after pool1)
H2P, W2P = H2 + 2, W2 + 2  # 34, 18
H3, W3 = 16, 8           # after pool2
NS = 25
BETA = 0.95
SCALE = float(2.0 ** 96)  # sigmoid saturation scale (exact power of two)

BC = 32                  # conv2 batch-chunk
NCHUNK = BL // BC        # 4
P1COLS = H2P * W2P * BC  # 34*18*32 = 19584
GROUPS = BL // 4         # 32 conv1 groups (4 batches each)

# spike engine split: tiles with index % MOD == 0 go to ACT, rest to DVE
ACT_MOD1 = 2
ACT_MOD2 = 2


def build_program():
    nc = bacc.Bacc()

    # ---- inputs (host-preprocessed layouts) ----
    XP = nc.declare_dram_parameter("xp", [BL, HP * WP], BF16, isOutput=False)
    WC1 = nc.declare_dram_parameter("wc1", [128, 128], BF16, isOutput=False)
    B1S = nc.declare_dram_parameter("b1s", [128, 1], F32, isOutput=False)   # -SCALE*thr1
    T1 = nc.declare_dram_parameter("t1", [128, 1], F32, isOutput=False)     # thr1
    WC2 = nc.declare_dram_parameter("wc2", [128, 576], BF16, isOutput=False)  # 4 reps x [32ch, 9 taps x 64]
    B2S = nc.declare_dram_parameter("b2s", [128, 1], F32, isOutput=False)   # -SCALE*thr2
    T2 = nc.declare_dram_parameter("t2", [128, 1], F32, isOutput=False)     # thr2
    FC1W = nc.declare_dram_parameter("fc1w", [2 * 64, 128 * 128], F32, isOutput=False)
    FC1B = nc.declare_dram_parameter("fc1b", [128, 2], F32, isOutput=False)
    FC2A = nc.declare_dram_parameter("fc2a", [128, 256], BF16, isOutput=False)  # lhsT hi [u3_low, (chunk, u4)]
    FC2B = nc.declare_dram_parameter("fc2b", [128, 256], BF16, isOutput=False)  # lo
    FC2BIAS = nc.declare_dram_parameter("fc2bias", [128, 1], F32, isOutput=False)
    FC3A = nc.declare_dram_parameter("fc3a", [128, 16], BF16, isOutput=False)   # lhsT hi [u4, 10->16]
    FC3B = nc.declare_dram_parameter("fc3b", [128, 16], BF16, isOutput=False)
    FC3BIAS = nc.declare_dram_parameter("fc3bias", [16, 1], F32, isOutput=False)

    OUT = nc.declare_dram_parameter("out", [16, NS * BL], F32, isOutput=True)

    dbg = os.environ.get("KERNEL_DEBUG", "")
    dbg_outs = {}
    if dbg:
        dbg_outs["p1"] = nc.declare_dram_parameter("dbg_p1", [128, P1COLS], BF16, isOutput=True)
        dbg_outs["c2"] = nc.declare_dram_parameter("dbg_c2", [128, 64 * BL], F32, isOutput=True)
        dbg_outs["cur3"] = nc.declare_dram_parameter("dbg_cur3", [256, BL], F32, isOutput=True)

    with tile.TileContext(nc) as tc, ExitStack() as top:
        consts = top.enter_context(tc.tile_pool(name="consts", bufs=1))

        wc1 = consts.tile([128, 128], BF16, tag="wc1")
        nc.sync.dma_start(wc1[:], WC1[:])
        b1s = consts.tile([128, 1], F32, tag="b1s")
        nc.sync.dma_start(b1s[:], B1S[:])
        t1 = consts.tile([128, 1], F32, tag="t1")
        nc.sync.dma_start(t1[:], T1[:])
        wc2 = consts.tile([128, 576], BF16, tag="wc2")
        nc.sync.dma_start(wc2[:], WC2[:])
        b2s = consts.tile([128, 1], F32, tag="b2s")
        nc.sync.dma_start(b2s[:], B2S[:])
        t2 = consts.tile([128, 1], F32, tag="t2")
        nc.sync.dma_start(t2[:], T2[:])

        # c2 accumulation buffer: [128 = 2x64ch, 64 rounds x 128 batch] fp32
        c2buf = top.enter_context(tc.tile_pool(name="c2bufp", bufs=1)).tile(
            [128, 64 * BL], F32, tag="c2buf")

        # ------------- conv1 + spike1 + pool1 + conv2 + spike2 + pool2 -------------
        with ExitStack() as convs:
            p1pool = convs.enter_context(tc.tile_pool(name="p1pool", bufs=2))
            xrep_pool = convs.enter_context(tc.tile_pool(name="xrep", bufs=3))
            s1pool = convs.enter_context(tc.tile_pool(name="s1pool", bufs=3))
            pwpool = convs.enter_context(tc.tile_pool(name="pwpool", bufs=2))
            phpool = convs.enter_context(tc.tile_pool(name="phpool", bufs=2))
            c1ps = convs.enter_context(tc.tile_pool(name="c1ps", bufs=2, space="PSUM"))
            c2ps = convs.enter_context(tc.tile_pool(name="c2ps", bufs=4, space="PSUM"))
            s2pool = convs.enter_context(tc.tile_pool(name="s2pool", bufs=4))
            pw2pool = convs.enter_context(tc.tile_pool(name="pw2pool", bufs=4))

            for chunk in range(NCHUNK):
                # P1: [128 = 4 replicas x 32ch, (h2p, w2p, BC)] bf16, pool-sums 0..4
                p1 = p1pool.tile([128, P1COLS], BF16, tag="p1")
                nc.gpsimd.memset(p1[0:32, :], 0.0)
                p1v = p1[:].rearrange("p (b h w) -> p b h w", b=BC, h=H2P, w=W2P)

                for gg in range(BC // 8):
                    # x_rep: [128 = 2 subgroups x (4b x 16taps-padded), 2048]
                    # tap rows 9..15 hold garbage; wc1 zero rows null them.
                    # xr rows: 64*sub + 4*k + b  (k = dy*4+dx over a 4x4 window)
                    xr = xrep_pool.tile([128, H * W], BF16, tag="xr")
                    xsrc = XP[:]
                    for sub_ in range(2):
                        for k in range(16):
                            dy, dx = k // 4, k % 4
                            src = bass.AP(xsrc.tensor,
                                          xsrc.offset + (chunk * BC + gg * 8 + 4 * sub_) * (HP * WP) + dy * WP + dx,
                                          [[HP * WP, 4], [WP, H], [1, W]])
                            r0 = 64 * sub_ + 4 * k
                            nc.sync.dma_start(xr[r0:r0 + 4, :], src)

                    for sub in range(2):
                        sb = 64 * sub
                        # conv1: 4 matmuls N=512 into 2 psum tiles [128, 1024]
                        s1 = s1pool.tile([128, H * W], BF16, tag="s1")
                        for half in range(2):
                            ps = c1ps.tile([128, 1024], F32, tag="c1ps")
                            for q in range(2):
                                off = half * 1024 + q * 512
                                nc.tensor.matmul(ps[:, q * 512:(q + 1) * 512],
                                                 wc1[sb:sb + 64, :],
                                                 xr[sb:sb + 64, off:off + 512],
                                                 start=True, stop=True,
                                                 tile_position=(sb, 0))
                            # spike1 -> {0,1} bf16
                            tgt = s1[:, half * 1024:(half + 1) * 1024]
                            if (2 * gg + sub + half) % ACT_MOD1 == 0:
                                nc.scalar.activation(tgt, ps[:], SIGMOID, bias=b1s[:], scale=SCALE)
                            else:
                                nc.vector.tensor_scalar(tgt, ps[:], t1[:], None, op0=GT)

                        # pool1 w-pairs: [128, 64h, 16w']
                        pw = pwpool.tile([128, H * (W // 2)], BF16, tag="pw")
                        s1v = s1[:].rearrange("p (h w2 t) -> p h w2 t", h=H, w2=W // 2, t=2)
                        pwv = pw[:].rearrange("p (h w2) -> p h w2", h=H)
                        nc.vector.tensor_add(pwv, s1v[:, :, :, 0], s1v[:, :, :, 1])
                        # pool1 h-pairs: [128, 32h', 16w']
                        ph = phpool.tile([128, (H // 2) * (W // 2)], BF16, tag="ph")
                        pwh = pw[:].rearrange("p (h2 t w2) -> p h2 t w2", h2=H // 2, t=2, w2=W // 2)
                        phv = ph[:].rearrange("p (h2 w2) -> p h2 w2", h2=H // 2)
                        nc.vector.tensor_add(phv, pwh[:, :, 0, :], pwh[:, :, 1, :])

                        # scatter into P1 replica 0 (interior), per batch
                        phb = ph[:].rearrange("p (h2 w2) -> p h2 w2", h2=H // 2)
                        for b4 in range(4):
                            bb = gg * 8 + sub * 4 + b4
                            dst = p1v[0:32, bb, 1:H2 + 1, 1:W2 + 1].squeeze()
                            nc.sync.dma_start(dst, phb[32 * b4:32 * (b4 + 1), :, :])

                # replicate P1 block 0 -> blocks 1..3
                for rep in range(1, 4):
                    nc.sync.dma_start(p1[32 * rep:32 * (rep + 1), :], p1[0:32, :])
                if dbg and chunk == 0:
                    nc.sync.dma_start(dbg_outs["p1"][:], p1[:])

                # ---- conv2 on this chunk ----
                c2q = c2buf[:].rearrange("p (j4 q b) -> p j4 q b", q=4, b=BL)
                pw2_prev = None
                for hrow in range(H2):         # h-rows
                    r = hrow % 4               # row-group slot
                    base = 32 * r
                    ps2 = c2ps.tile([64, W2 * BC], F32, tag="c2ps")
                    for k in range(9):
                        dy, dx = k // 3, k % 3
                        w_ap = wc2[base:base + 32, 64 * k:64 * (k + 1)]
                        rhs = p1v[base:base + 32, :, hrow + dy, dx:dx + W2]
                        nc.tensor.matmul(ps2[:], w_ap, rhs,
                                         start=(k == 0), stop=(k == 8),
                                         tile_position=(base, 0))
                    # spike2 -> {0,1} bf16
                    s2 = s2pool.tile([64, W2 * BC], BF16, tag="s2")
                    if hrow % ACT_MOD2 == 0:
                        nc.scalar.activation(s2[:], ps2[:], SIGMOID, bias=b2s[0:64, :], scale=SCALE)
                    else:
                        nc.vector.tensor_scalar(s2[:], ps2[:], t2[0:64, :], None, op0=GT)
                    # pool2 w-pairs -> [64, (BC, 8w')]
                    pw2 = pw2pool.tile([64, (W2 // 2) * BC], BF16, tag="pw2",
                                       name=f"pw2_{chunk}_{hrow}")
                    s2v = s2[:].rearrange("p (b w2 t) -> p b w2 t", b=BC, w2=W2 // 2, t=2)
                    pw2v = pw2[:].rearrange("p (b w2) -> p b w2", b=BC)
                    nc.vector.tensor_add(pw2v, s2v[:, :, :, 0], s2v[:, :, :, 1])
                    if hrow % 2 == 0:
                        pw2_prev = pw2
                        continue
                    # pool2 h-pairs -> c2buf strided, fp32
                    # even w' -> partitions 0:64, odd w' -> partitions 64:128
                    j = hrow // 2
                    pa = pw2_prev[:].rearrange("p (b w4 t) -> p b w4 t", b=BC, w4=W2 // 4, t=2)
                    pb = pw2[:].rearrange("p (b w4 t) -> p b w4 t", b=BC, w4=W2 // 4, t=2)
                    for par in range(2):
                        dst = c2q[64 * par:64 * (par + 1), j,
                                  :, chunk * BC:(chunk + 1) * BC].squeeze()
                        dst = dst.transpose([0, 2, 1])
                        nc.vector.tensor_add(dst, pa[:, :, :, par], pb[:, :, :, par])

        if dbg:
            nc.sync.dma_start(dbg_outs["c2"][:], c2buf[:])

        # ---------------- fc1 (fp32) + LIF ----------------
        with ExitStack() as fcs:
            fc1wpool = fcs.enter_context(tc.tile_pool(name="fc1w", bufs=6))
            fc1ps = fcs.enter_context(tc.tile_pool(name="fc1ps", bufs=1, space="PSUM"))
            lifps = fcs.enter_context(tc.tile_pool(name="lifps", bufs=2, space="PSUM"))
            lifc = fcs.enter_context(tc.tile_pool(name="lifc", bufs=1))

            fc1b = consts.tile([128, 2], F32, tag="fc1b")
            nc.sync.dma_start(fc1b[:], FC1B[:])

            cur3 = [lifc.tile([128, BL], F32, tag=f"cur3_{h}", name=f"cur3_{h}") for h in range(2)]
            for h in range(2):
                pA = fc1ps.tile([128, BL], F32, tag="fc1psA")
                pB = fc1ps.tile([128, BL], F32, tag="fc1psB")
                c2r = c2buf[:].rearrange("p (r b) -> p r b", b=BL)
                for r in range(64):
                    wt = fc1wpool.tile([128, 128], F32, tag="fc1wt")
                    nc.sync.dma_start(wt[:], FC1W[64 * h + r, :].rearrange("(p m) -> p m", p=128))
                    nc.tensor.matmul(pA[:], wt[0:64, :], c2r[0:64, r, :],
                                     start=(r == 0), stop=(r == 63), tile_position=(0, 0))
                    nc.tensor.matmul(pB[:], wt[64:128, :], c2r[64:128, r, :],
                                     start=(r == 0), stop=(r == 63), tile_position=(64, 0))
                tsum = lifc.tile([128, BL], F32, tag=f"tsum_{h}", name=f"tsum_{h}")
                # cur3 = 0.25*(pA + pB) + fc1_b, one PSUM input per op
                nc.vector.tensor_scalar(tsum[:], pA[:], 0.25, fc1b[:, h:h + 1],
                                        op0=MUL, op1=ADD)
                nc.vector.scalar_tensor_tensor(cur3[h][:], pB[:], 0.25, tsum[:],
                                               op0=MUL, op1=ADD)
            if dbg:
                nc.sync.dma_start(dbg_outs["cur3"][0:128, :], cur3[0][:])
                nc.sync.dma_start(dbg_outs["cur3"][128:256, :], cur3[1][:])

            # LIF state + weights
            fc2a = consts.tile([128, 256], BF16, tag="fc2a")
            nc.sync.dma_start(fc2a[:], FC2A[:])
            fc2b_w = consts.tile([128, 256], BF16, tag="fc2b_w")
            nc.sync.dma_start(fc2b_w[:], FC2B[:])
            fc2bias = consts.tile([128, 1], F32, tag="fc2bias")
            nc.sync.dma_start(fc2bias[:], FC2BIAS[:])
            fc3a = consts.tile([128, 16], BF16, tag="fc3a")
            nc.sync.dma_start(fc3a[:], FC3A[:])
            fc3b_w = consts.tile([128, 16], BF16, tag="fc3b_w")
            nc.sync.dma_start(fc3b_w[:], FC3B[:])
            fc3bias = consts.tile([16, 1], F32, tag="fc3bias")
            nc.sync.dma_start(fc3bias[:], FC3BIAS[:])

            mem3 = [lifc.tile([128, BL], F32, tag=f"mem3_{h}", name=f"mem3_{h}") for h in range(2)]
            spk3 = [lifc.tile([128, BL], BF16, tag=f"spk3_{h}", name=f"spk3_{h}") for h in range(2)]
            mem4 = lifc.tile([128, BL], F32, tag="mem4")
            spk4 = lifc.tile([128, BL], BF16, tag="spk4")
            mem5 = lifc.tile([16, BL], F32, tag="mem5")
            spk5 = lifc.tile([16, BL], F32, tag="spk5")
            outstage = lifc.tile([16, NS * BL], F32, tag="outstage")
            for t_ in mem3 + spk3 + [mem4, spk4, mem5, spk5]:
                nc.vector.memset(t_[:], 0.0)
            nc.vector.memset(outstage[:], 0.0)

            t3 = [lifc.tile([128, BL], F32, tag=f"t3_{h}", name=f"t3_{h}") for h in range(2)]
            t4 = lifc.tile([128, BL], F32, tag="t4")
            t5 = lifc.tile([16, BL], F32, tag="t5")

            for step in range(NS):
                # layer 3 (cur3 constant across steps)
                for h in range(2):
                    nc.vector.scalar_tensor_tensor(t3[h][:], mem3[h][:], BETA, cur3[h][:],
                                                   op0=MUL, op1=ADD)
                    nc.vector.tensor_sub(mem3[h][:], t3[h][:], spk3[h][:])
                    nc.vector.tensor_scalar(spk3[h][:], mem3[h][:], 1.0, None, op0=GT)
                # fc2 (hi/lo split)
                p4 = lifps.tile([128, BL], F32, tag="p4")
                nc.tensor.matmul(p4[:], fc2a[:, 0:128], spk3[0][:], start=True, stop=False)
                nc.tensor.matmul(p4[:], fc2a[:, 128:256], spk3[1][:], start=False, stop=False)
                nc.tensor.matmul(p4[:], fc2b_w[:, 0:128], spk3[0][:], start=False, stop=False)
                nc.tensor.matmul(p4[:], fc2b_w[:, 128:256], spk3[1][:], start=False, stop=True)
                # layer 4
                nc.vector.tensor_scalar(t4[:], mem4[:], BETA, fc2bias[:], op0=MUL, op1=ADD)
                nc.vector.tensor_add(t4[:], t4[:], p4[:])
                nc.vector.tensor_sub(mem4[:], t4[:], spk4[:])
                nc.vector.tensor_scalar(spk4[:], mem4[:], 1.0, None, op0=GT)
                # fc3
                p5 = lifps.tile([16, BL], F32, tag="p5")
                nc.tensor.matmul(p5[:], fc3a[:], spk4[:], start=True, stop=False)
                nc.tensor.matmul(p5[:], fc3b_w[:], spk4[:], start=False, stop=True)
                # layer 5
                nc.vector.tensor_scalar(t5[:], mem5[:], BETA, fc3bias[:], op0=MUL, op1=ADD)
                nc.vector.tensor_add(t5[:], t5[:], p5[:])
                nc.vector.tensor_sub(mem5[:], t5[:], spk5[:])
                nc.vector.tensor_scalar(spk5[:], mem5[:], 1.0, None, op0=GT)
                nc.vector.tensor_copy(outstage[:, step * BL:(step + 1) * BL], spk5[:])

            nc.sync.dma_start(OUT[:], outstage[:])

    nc.compile()
    return nc


def _prep_inputs(x, conv1_w, conv1_b, conv2_w, conv2_b, fc1_w, fc1_b,
                 fc2_w, fc2_b, fc3_w, fc3_b):
    """Host-side preprocessing -> list of 8 per-core input dicts."""
    bf = ml_dtypes.bfloat16

    # conv1 weights: [128, 128]: 2 replicas of block-diag [64 = 4b x 16taps, 128]
    wc1 = np.zeros((128, 128), np.float32)
    w1 = conv1_w.reshape(32, 3, 3)  # [c, dy, dx]
    for sub in range(2):
        for dy in range(3):
            for dx in range(3):
                k = 4 * dy + dx
                for b4 in range(4):
                    wc1[64 * sub + 4 * k + b4, 32 * b4:32 * (b4 + 1)] = w1[:, dy, dx]
    wc1 = wc1.astype(bf)

    thr1 = (1.0 - conv1_b).astype(np.float32)          # [32]
    t1 = np.tile(thr1, 4).reshape(128, 1).astype(np.float32)
    b1s = (-(t1.astype(np.float64)) * SCALE).astype(np.float32)

    # conv2 weights: [128 = 4 reps x 32ch_in, 9 taps x 64 ch_out]
    wtap = conv2_w.transpose(1, 2, 3, 0).reshape(32, 9, 64)  # [c_in, k, c_out]
    wc2_one = wtap.reshape(32, 576)
    wc2 = np.tile(wc2_one, (4, 1)).astype(bf)
    thr2 = (4.0 * (1.0 - conv2_b)).astype(np.float32)  # [64]
    t2 = np.tile(thr2, 2).reshape(128, 1).astype(np.float32)
    b2s = (-(t2.astype(np.float64)) * SCALE).astype(np.float32)

    # fc1 weights: [2*64 rounds, 128 part * 128 units] fp32
    # part p: ch = p%64, hw' = 2r + p//64; feat = ch*128 + hw'; unit = 128h + m
    fw = fc1_w.reshape(256, 64, 128)       # [u, ch, hw']
    fc1wt = np.zeros((2, 64, 128, 128), np.float32)
    for h in range(2):
        us = fw[128 * h:128 * (h + 1)]     # [128u, 64ch, 128hw]
        for r in range(64):
            fc1wt[h, r, 0:64, :] = us[:, :, 2 * r].T       # [ch, u]
            fc1wt[h, r, 64:128, :] = us[:, :, 2 * r + 1].T
    fc1wt = fc1wt.reshape(2 * 64, 128 * 128)
    fc1b = np.ascontiguousarray(fc1_b.reshape(2, 128).T).astype(np.float32)

    # fc2: lhsT [u3, u4]; hi/lo split
    l2 = np.ascontiguousarray(fc2_w.T).astype(np.float32)   # [256 u3, 128 u4]
    l2a_full = l2.astype(bf)
    l2b_full = (l2 - l2a_full.astype(np.float32)).astype(bf)
    def chunked(a):  # [256, 128] -> [128, 256] with chunk-major cols
        return np.ascontiguousarray(a.reshape(2, 128, 128).transpose(1, 0, 2).reshape(128, 256))
    l2a = chunked(l2a_full)
    l2b = chunked(l2b_full)
    fc2bias = fc2_b.reshape(128, 1).astype(np.float32)

    l3 = np.zeros((128, 16), np.float32)
    l3[:, 0:10] = fc3_w.T                  # [u4, 10]
    l3a = l3.astype(bf)
    l3b = (l3 - l3a.astype(np.float32)).astype(bf)
    fc3bias = np.zeros((16, 1), np.float32)
    fc3bias[0:10, 0] = fc3_b

    common = dict(wc1=wc1, b1s=b1s, t1=t1, wc2=wc2, b2s=b2s, t2=t2,
                  fc1w=fc1wt, fc1b=fc1b, fc2a=l2a, fc2b=l2b, fc2bias=fc2bias,
                  fc3a=l3a, fc3b=l3b, fc3bias=fc3bias)

    # x: pad to [BL, 66, 34] bf16 per core
    xs = x.reshape(B, H, W)
    in_maps = []
    for c in range(N_CORES):
        xc = xs[c * BL:(c + 1) * BL]
        xp = np.zeros((BL, HP, WP), np.float32)
        xp[:, 1:H + 1, 1:W + 1] = xc
        m = dict(common)
        m["xp"] = xp.reshape(BL, HP * WP).astype(bf)
        in_maps.append(m)
    return in_maps


_NC_CACHE = {}


def _get_nc():
    if "nc" not in _NC_CACHE:
        _NC_CACHE["nc"] = build_program()
    return _NC_CACHE["nc"]


def kernel(**inputs):
    nc = _get_nc()
    in_maps = _prep_inputs(**inputs)
    res = run_bass_kernel_spmd(nc, in_maps, core_ids=list(range(N_CORES)))
    outs = []
    for c in range(N_CORES):
        o = res.results[c]["out"]            # [16, NS*BL]
        o = o.reshape(16, NS, BL)[0:10]      # [10, NS, BL]
        outs.append(o.transpose(1, 2, 0))    # [NS, BL, 10]
    return np.concatenate(outs, axis=1).astype(np.float32)  # [NS, B, 10]


# revision 16
# speedup vs baseline: 1.2837x; 1.2837x over previous
"""AudioSNN forward pass on 8 Trainium2 NeuronCores (pure data parallel).

Host side: weight re-layout + padding (numpy). Device side: Bass/Tile kernel
per core over a 128-batch shard.
"""
import os
import sys
import numpy as np

for _p in ("/opt/trn_rl_repo", "/root/.axon_site/_ro/trn_rl_repo"):
    if os.path.isdir(_p) and _p not in sys.path:
        sys.path.insert(0, _p)

import ml_dtypes
from contextlib import ExitStack

import concourse.bass as bass
import concourse.tile as tile
from concourse import mybir, bacc
from concourse.bass_utils import run_bass_kernel_spmd

BF16 = mybir.dt.bfloat16
F32 = mybir.dt.float32
GT = mybir.AluOpType.is_gt
ADD = mybir.AluOpType.add
SUB = mybir.AluOpType.subtract
MUL = mybir.AluOpType.mult
SIGMOID = mybir.ActivationFunctionType.Sigmoid

N_CORES = 8
B = 1024
BL = B // N_CORES        # 128 batch per core
H, W = 64, 32            # conv1 image
HP, WP = H + 3, W + 3    # padded (67, 35); conv1 uses a 4x4 tap window
H2, W2 = 32, 16          # conv2 image (after pool1)
H2P, W2P = H2 + 2, W2 + 2  # 34, 18
H3, W3 = 16, 8           # after pool2
NS = 25
BETA = 0.95
SCALE = float(2.0 ** 96)  # sigmoid saturation scale (exact power of two)

BC = 32                  # conv2 batch-chunk
NCHUNK = BL // BC        # 4
P1COLS = H2P * W2P * BC  # 34*18*32 = 19584
GROUPS = BL // 4         # 32 conv1 groups (4 batches each)

# spike engine split: tiles with index % MOD == 0 go to ACT, rest to DVE
ACT_MOD1 = 2
ACT_MOD2 = 2


def build_program():
    nc = bacc.Bacc()

    # ---- inputs (host-preprocessed layouts) ----
    XP = nc.declare_dram_parameter("xp", [BL + 1, HP * WP], BF16, isOutput=False)
    WC1 = nc.declare_dram_parameter("wc1", [128, 128], BF16, isOutput=False)
    B1S = nc.declare_dram_parameter("b1s", [128, 1], F32, isOutput=False)   # -SCALE*thr1
    T1 = nc.declare_dram_parameter("t1", [128, 1], F32, isOutput=False)     # thr1
    WC2 = nc.declare_dram_parameter("wc2", [128, 576], BF16, isOutput=False)  # 4 reps x [32ch, 9 taps x 64]
    B2S = nc.declare_dram_parameter("b2s", [128, 1], F32, isOutput=False)   # -SCALE*thr2
    T2 = nc.declare_dram_parameter("t2", [128, 1], F32, isOutput=False)     # thr2
    FC1W = nc.declare_dram_parameter("fc1w", [2 * 64, 128 * 128], F32, isOutput=False)
    FC1B = nc.declare_dram_parameter("fc1b", [128, 2], F32, isOutput=False)
    FC2A = nc.declare_dram_parameter("fc2a", [128, 256], BF16, isOutput=False)  # lhsT hi [u3_low, (chunk, u4)]
    FC2B = nc.declare_dram_parameter("fc2b", [128, 256], BF16, isOutput=False)  # lo
    FC2BIAS = nc.declare_dram_parameter("fc2bias", [128, 1], F32, isOutput=False)
    FC3A = nc.declare_dram_parameter("fc3a", [128, 16], BF16, isOutput=False)   # lhsT hi [u4, 10->16]
    FC3B = nc.declare_dram_parameter("fc3b", [128, 16], BF16, isOutput=False)
    FC3BIAS = nc.declare_dram_parameter("fc3bias", [16, 1], F32, isOutput=False)

    OUT = nc.declare_dram_parameter("out", [16, NS * BL], F32, isOutput=True)

    dbg = os.environ.get("KERNEL_DEBUG", "")
    dbg_outs = {}
    if dbg:
        dbg_outs["p1"] = nc.declare_dram_parameter("dbg_p1", [128, P1COLS], BF16, isOutput=True)
        dbg_outs["c2"] = nc.declare_dram_parameter("dbg_c2", [128, 64 * BL], F32, isOutput=True)
        dbg_outs["cur3"] = nc.declare_dram_parameter("dbg_cur3", [256, BL], F32, isOutput=True)

    with tile.TileContext(nc) as tc, ExitStack() as top:
        consts = top.enter_context(tc.tile_pool(name="consts", bufs=1))

        wc1 = consts.tile([128, 128], BF16, tag="wc1")
        nc.sync.dma_start(wc1[:], WC1[:])
        b1s = consts.tile([128, 1], F32, tag="b1s")
        nc.sync.dma_start(b1s[:], B1S[:])
        t1 = consts.tile([128, 1], F32, tag="t1")
        nc.sync.dma_start(t1[:], T1[:])
        wc2 = consts.tile([128, 576], BF16, tag="wc2")
        nc.sync.dma_start(wc2[:], WC2[:])
        b2s = consts.tile([128, 1], F32, tag="b2s")
        nc.sync.dma_start(b2s[:], B2S[:])
        t2 = consts.tile([128, 1], F32, tag="t2")
        nc.sync.dma_start(t2[:], T2[:])

        # c2 accumulation buffer: [128 = 2x64ch, 64 rounds x 128 batch] fp32
        c2buf = top.enter_context(tc.tile_pool(name="c2bufp", bufs=1)).tile(
            [128, 64 * BL], F32, tag="c2buf")

        # ------------- conv1 + spike1 + pool1 + conv2 + spike2 + pool2 -------------
        with ExitStack() as convs:
            p1pool = convs.enter_context(tc.tile_pool(name="p1pool", bufs=1))
            xrep_pool = convs.enter_context(tc.tile_pool(name="xrep", bufs=3))
            s1pool = convs.enter_context(tc.tile_pool(name="s1pool", bufs=3))
            pwpool = convs.enter_context(tc.tile_pool(name="pwpool", bufs=2))
            stgpool = convs.enter_context(tc.tile_pool(name="stgpool", bufs=1))
            c1ps = convs.enter_context(tc.tile_pool(name="c1ps", bufs=2, space="PSUM"))
            c2ps = convs.enter_context(tc.tile_pool(name="c2ps", bufs=4, space="PSUM"))
            s2pool = convs.enter_context(tc.tile_pool(name="s2pool", bufs=4))
            pw2pool = convs.enter_context(tc.tile_pool(name="pw2pool", bufs=4))

            # persistent double-buffered P1 + padded pool1 staging; pads are
            # zeroed once and never overwritten afterwards.
            p1bufs = [p1pool.tile([128, P1COLS], BF16, tag=f"p1_{i}", name=f"p1_{i}")
                      for i in range(2)]
            for i in range(2):
                nc.gpsimd.memset(p1bufs[i][0:32, :], 0.0)
            stages = [stgpool.tile([128, H2 * W2P], BF16, tag=f"stg_{i}", name=f"stg_{i}")
                      for i in range(3)]
            for i in range(3):
                nc.vector.memset(stages[i][:], 0.0)

            for chunk in range(NCHUNK):
                # P1: [128 = 4 replicas x 32ch, (b, h2p, w2p)] bf16, pool-sums 0..4
                p1 = p1bufs[chunk % 2]
                p1v = p1[:].rearrange("p (b h w) -> p b h w", b=BC, h=H2P, w=W2P)

                for gg in range(BC // 8):
                    # xr rows: 64*sub + 4*k + b; each row holds a contiguous
                    # 64x35 window of padded x starting at (dy, dx)
                    XW = H * WP  # 2240
                    xr = xrep_pool.tile([128, XW], BF16, tag="xr")
                    xrv = xr[:].rearrange("p (h w) -> p h w", h=H, w=WP)
                    xsrc = XP[:]
                    for sub_ in range(2):
                        for k in range(16):
                            dy, dx = k // 4, k % 4
                            src = bass.AP(xsrc.tensor,
                                          xsrc.offset + (chunk * BC + gg * 8 + 4 * sub_) * (HP * WP) + dy * WP + dx,
                                          [[HP * WP, 4], [1, XW]])
                            r0 = 64 * sub_ + 4 * k
                            nc.sync.dma_start(xr[r0:r0 + 4, :], src)

                    for sub in range(2):
                        sb = 64 * sub
                        # conv1: 4 matmuls N=512 into 2 psum tiles [128, 1024]
                        s1 = s1pool.tile([128, H * W], BF16, tag="s1")
                        for half in range(2):
                            ps = c1ps.tile([128, 1024], F32, tag="c1ps")
                            for q in range(2):
                                h0 = 16 * (2 * half + q)
                                nc.tensor.matmul(ps[:, q * 512:(q + 1) * 512],
                                                 wc1[sb:sb + 64, :],
                                                 xrv[sb:sb + 64, h0:h0 + 16, 0:W],
                                                 start=True, stop=True,
                                                 tile_position=(sb, 0))
                            # spike1 -> {0,1} bf16
                            tgt = s1[:, half * 1024:(half + 1) * 1024]
                            if (2 * gg + sub + half) % ACT_MOD1 == 0:
                                nc.scalar.activation(tgt, ps[:], SIGMOID, bias=b1s[:], scale=SCALE)
                            else:
                                nc.vector.tensor_scalar(tgt, ps[:], t1[:], None, op0=GT)

                        # pool1 w-pairs: [128, 64h, 16w']
                        pw = pwpool.tile([128, H * (W // 2)], BF16, tag="pw")
                        s1v = s1[:].rearrange("p (h w2 t) -> p h w2 t", h=H, w2=W // 2, t=2)
                        pwv = pw[:].rearrange("p (h w2) -> p h w2", h=H)
                        nc.vector.tensor_add(pwv, s1v[:, :, :, 0], s1v[:, :, :, 1])
                        # pool1 h-pairs into padded staging [128, (32h, 18w)]
                        stg = stages[(gg * 2 + sub) % 3]
                        sv = stg[:].rearrange("p (h2 v) -> p h2 v", h2=H2, v=W2P)
                        pwh = pw[:].rearrange("p (h2 t w2) -> p h2 t w2", h2=H // 2, t=2, w2=W // 2)
                        nc.vector.tensor_add(sv[:, :, 1:W2 + 1], pwh[:, :, 0, :], pwh[:, :, 1, :])

                        # scatter into P1 replica 0 (interior rows), per batch
                        for b4 in range(4):
                            bb = gg * 8 + sub * 4 + b4
                            dst = bass.AP(p1.tensor, p1.offset + bb * (H2P * W2P) + W2P,
                                          [[P1COLS, 32], [1, H2 * W2P]])
                            nc.sync.dma_start(dst, stg[32 * b4:32 * (b4 + 1), :])

                # replicate P1 block 0 -> blocks 1..3
                for rep in range(1, 4):
                    nc.sync.dma_start(p1[32 * rep:32 * (rep + 1), :], p1[0:32, :])
                if dbg and chunk == 0:
                    nc.sync.dma_start(dbg_outs["p1"][:], p1[:])

                # ---- conv2 on this chunk ----
                c2q = c2buf[:].rearrange("p (j4 q b) -> p j4 q b", q=4, b=BL)
                pw2_prev = None
                for hrow in range(H2):         # h-rows
                    r = hrow % 4               # row-group slot
                    base = 32 * r
                    ps2 = c2ps.tile([64, W2 * BC], F32, tag="c2ps")
                    for k in range(9):
                        dy, dx = k // 3, k % 3
                        w_ap = wc2[base:base + 32, 64 * k:64 * (k + 1)]
                        rhs = p1v[base:base + 32, :, hrow + dy, dx:dx + W2]
                        nc.tensor.matmul(ps2[:], w_ap, rhs,
                                         start=(k == 0), stop=(k == 8),
                                         tile_position=(base, 0))
                    # spike2 -> {0,1} bf16
                    s2 = s2pool.tile([64, W2 * BC], BF16, tag="s2")
                    if hrow % ACT_MOD2 == 0:
                        nc.scalar.activation(s2[:], ps2[:], SIGMOID, bias=b2s[0:64, :], scale=SCALE)
                    else:
                        nc.vector.tensor_scalar(s2[:], ps2[:], t2[0:64, :], None, op0=GT)
                    # pool2 w-pairs -> [64, (BC, 8w')]
                    pw2 = pw2pool.tile([64, (W2 // 2) * BC], BF16, tag="pw2",
                                       name=f"pw2_{chunk}_{hrow}")
                    s2v = s2[:].rearrange("p (b w2 t) -> p b w2 t", b=BC, w2=W2 // 2, t=2)
                    pw2v = pw2[:].rearrange("p (b w2) -> p b w2", b=BC)
                    nc.vector.tensor_add(pw2v, s2v[:, :, :, 0], s2v[:, :, :, 1])
                    if hrow % 2 == 0:
                        pw2_prev = pw2
                        continue
                    # pool2 h-pairs -> c2buf strided, fp32
                    # even w' -> partitions 0:64, odd w' -> partitions 64:128
                    j = hrow // 2
                    pa = pw2_prev[:].rearrange("p (b w4 t) -> p b w4 t", b=BC, w4=W2 // 4, t=2)
                    pb = pw2[:].rearrange("p (b w4 t) -> p b w4 t", b=BC, w4=W2 // 4, t=2)
                    for par in range(2):
                        dst = c2q[64 * par:64 * (par + 1), j,
                                  :, chunk * BC:(chunk + 1) * BC].squeeze()
                        dst = dst.transpose([0, 2, 1])
                        nc.vector.tensor_add(dst, pa[:, :, :, par], pb[:, :, :, par])

        if dbg:
            nc.sync.dma_start(dbg_outs["c2"][:], c2buf[:])

        # ---------------- fc1 (fp32) + LIF ----------------
        with ExitStack() as fcs:
            fc1wpool = fcs.enter_context(tc.tile_pool(name="fc1w", bufs=6))
            fc1ps = fcs.enter_context(tc.tile_pool(name="fc1ps", bufs=1, space="PSUM"))
            lifps = fcs.enter_context(tc.tile_pool(name="lifps", bufs=2, space="PSUM"))
            lifc = fcs.enter_context(tc.tile_pool(name="lifc", bufs=1))

            fc1b = consts.tile([128, 2], F32, tag="fc1b")
            nc.sync.dma_start(fc1b[:], FC1B[:])

            cur3 = [lifc.tile([128, BL], F32, tag=f"cur3_{h}", name=f"cur3_{h}") for h in range(2)]
            for h in range(2):
                pA = fc1ps.tile([128, BL], F32, tag="fc1psA")
                pB = fc1ps.tile([128, BL], F32, tag="fc1psB")
                c2r = c2buf[:].rearrange("p (r b) -> p r b", b=BL)
                for r in range(64):
                    wt = fc1wpool.tile([128, 128], F32, tag="fc1wt")
                    nc.sync.dma_start(wt[:], FC1W[64 * h + r, :].rearrange("(p m) -> p m", p=128))
                    nc.tensor.matmul(pA[:], wt[0:64, :], c2r[0:64, r, :],
                                     start=(r == 0), stop=(r == 63), tile_position=(0, 0))
                    nc.tensor.matmul(pB[:], wt[64:128, :], c2r[64:128, r, :],
                                     start=(r == 0), stop=(r == 63), tile_position=(64, 0))
                tsum = lifc.tile([128, BL], F32, tag=f"tsum_{h}", name=f"tsum_{h}")
                # cur3 = 0.25*(pA + pB) + fc1_b, one PSUM input per op
                nc.vector.tensor_scalar(tsum[:], pA[:], 0.25, fc1b[:, h:h + 1],
                                        op0=MUL, op1=ADD)
                nc.vector.scalar_tensor_tensor(cur3[h][:], pB[:], 0.25, tsum[:],
                                               op0=MUL, op1=ADD)
            if dbg:
                nc.sync.dma_start(dbg_outs["cur3"][0:128, :], cur3[0][:])
                nc.sync.dma_start(dbg_outs["cur3"][128:256, :], cur3[1][:])

            # LIF state + weights
            fc2a = consts.tile([128, 256], BF16, tag="fc2a")
            nc.sync.dma_start(fc2a[:], FC2A[:])
            fc2b_w = consts.tile([128, 256], BF16, tag="fc2b_w")
            nc.sync.dma_start(fc2b_w[:], FC2B[:])
            fc2bias = consts.tile([128, 1], F32, tag="fc2bias")
            nc.sync.dma_start(fc2bias[:], FC2BIAS[:])
            fc3a = consts.tile([128, 16], BF16, tag="fc3a")
            nc.sync.dma_start(fc3a[:], FC3A[:])
            fc3b_w = consts.tile([128, 16], BF16, tag="fc3b_w")
            nc.sync.dma_start(fc3b_w[:], FC3B[:])
            fc3bias = consts.tile([16, 1], F32, tag="fc3bias")
            nc.sync.dma_start(fc3bias[:], FC3BIAS[:])

            mem3 = [lifc.tile([128, BL], F32, tag=f"mem3_{h}", name=f"mem3_{h}") for h in range(2)]
            spk3 = [lifc.tile([128, BL], BF16, tag=f"spk3_{h}", name=f"spk3_{h}") for h in range(2)]
            mem4 = lifc.tile([128, BL], F32, tag="mem4")
            spk4 = lifc.tile([128, BL], BF16, tag="spk4")
            mem5 = lifc.tile([16, BL], F32, tag="mem5")
            spk5 = lifc.tile([16, BL], F32, tag="spk5")
            outstage = lifc.tile([16, NS * BL], F32, tag="outstage")
            for t_ in mem3 + spk3 + [mem4, spk4, mem5, spk5]:
                nc.vector.memset(t_[:], 0.0)
            nc.vector.memset(outstage[:], 0.0)

            t3 = [lifc.tile([128, BL], F32, tag=f"t3_{h}", name=f"t3_{h}") for h in range(2)]
            t4 = lifc.tile([128, BL], F32, tag="t4")
            t5 = lifc.tile([16, BL], F32, tag="t5")

            for step in range(NS):
                # layer 3 (cur3 constant across steps)
                for h in range(2):
                    nc.vector.scalar_tensor_tensor(t3[h][:], mem3[h][:], BETA, cur3[h][:],
                                                   op0=MUL, op1=ADD)
                    nc.vector.tensor_sub(mem3[h][:], t3[h][:], spk3[h][:])
                    nc.vector.tensor_scalar(spk3[h][:], mem3[h][:], 1.0, None, op0=GT)
                # fc2 (hi/lo split)
                p4 = lifps.tile([128, BL], F32, tag="p4")
                nc.tensor.matmul(p4[:], fc2a[:, 0:128], spk3[0][:], start=True, stop=False)
                nc.tensor.matmul(p4[:], fc2a[:, 128:256], spk3[1][:], start=False, stop=False)
                nc.tensor.matmul(p4[:], fc2b_w[:, 0:128], spk3[0][:], start=False, stop=False)
                nc.tensor.matmul(p4[:], fc2b_w[:, 128:256], spk3[1][:], start=False, stop=True)
                # layer 4
                nc.vector.tensor_scalar(t4[:], mem4[:], BETA, fc2bias[:], op0=MUL, op1=ADD)
                nc.vector.tensor_add(t4[:], t4[:], p4[:])
                nc.vector.tensor_sub(mem4[:], t4[:], spk4[:])
                nc.vector.tensor_scalar(spk4[:], mem4[:], 1.0, None, op0=GT)
                # fc3
                p5 = lifps.tile([16, BL], F32, tag="p5")
                nc.tensor.matmul(p5[:], fc3a[:], spk4[:], start=True, stop=False)
                nc.tensor.matmul(p5[:], fc3b_w[:], spk4[:], start=False, stop=True)
                # layer 5
                nc.vector.tensor_scalar(t5[:], mem5[:], BETA, fc3bias[:], op0=MUL, op1=ADD)
                nc.vector.tensor_add(t5[:], t5[:], p5[:])
                nc.vector.tensor_sub(mem5[:], t5[:], spk5[:])
                nc.vector.tensor_scalar(spk5[:], mem5[:], 1.0, None, op0=GT)
                nc.vector.tensor_copy(outstage[:, step * BL:(step + 1) * BL], spk5[:])

            nc.sync.dma_start(OUT[:], outstage[:])

    nc.compile()
    return nc


def _prep_inputs(x, conv1_w, conv1_b, conv2_w, conv2_b, fc1_w, fc1_b,
                 fc2_w, fc2_b, fc3_w, fc3_b):
    """Host-side preprocessing -> list of 8 per-core input dicts."""
    bf = ml_dtypes.bfloat16

    # conv1 weights: [128, 128]: 2 replicas of block-diag [64 = 4b x 16taps, 128]
    wc1 = np.zeros((128, 128), np.float32)
    w1 = conv1_w.reshape(32, 3, 3)  # [c, dy, dx]
    for sub in range(2):
        for dy in range(3):
            for dx in range(3):
                k = 4 * dy + dx
                for b4 in range(4):
                    wc1[64 * sub + 4 * k + b4, 32 * b4:32 * (b4 + 1)] = w1[:, dy, dx]
    wc1 = wc1.astype(bf)

    thr1 = (1.0 - conv1_b).astype(np.float32)          # [32]
    t1 = np.tile(thr1, 4).reshape(128, 1).astype(np.float32)
    b1s = (-(t1.astype(np.float64)) * SCALE).astype(np.float32)

    # conv2 weights: [128 = 4 reps x 32ch_in, 9 taps x 64 ch_out]
    wtap = conv2_w.transpose(1, 2, 3, 0).reshape(32, 9, 64)  # [c_in, k, c_out]
    wc2_one = wtap.reshape(32, 576)
    wc2 = np.tile(wc2_one, (4, 1)).astype(bf)
    thr2 = (4.0 * (1.0 - conv2_b)).astype(np.float32)  # [64]
    t2 = np.tile(thr2, 2).reshape(128, 1).astype(np.float32)
    b2s = (-(t2.astype(np.float64)) * SCALE).astype(np.float32)

    # fc1 weights: [2*64 rounds, 128 part * 128 units] fp32
    # part p: ch = p%64, hw' = 2r + p//64; feat = ch*128 + hw'; unit = 128h + m
    fw = fc1_w.reshape(256, 64, 128)       # [u, ch, hw']
    fc1wt = np.zeros((2, 64, 128, 128), np.float32)
    for h in range(2):
        us = fw[128 * h:128 * (h + 1)]     # [128u, 64ch, 128hw]
        for r in range(64):
            fc1wt[h, r, 0:64, :] = us[:, :, 2 * r].T       # [ch, u]
            fc1wt[h, r, 64:128, :] = us[:, :, 2 * r + 1].T
    fc1wt = fc1wt.reshape(2 * 64, 128 * 128)
    fc1b = np.ascontiguousarray(fc1_b.reshape(2, 128).T).astype(np.float32)

    # fc2: lhsT [u3, u4]; hi/lo split
    l2 = np.ascontiguousarray(fc2_w.T).astype(np.float32)   # [256 u3, 128 u4]
    l2a_full = l2.astype(bf)
    l2b_full = (l2 - l2a_full.astype(np.float32)).astype(bf)
    def chunked(a):  # [256, 128] -> [128, 256] with chunk-major cols
        return np.ascontiguousarray(a.reshape(2, 128, 128).transpose(1, 0, 2).reshape(128, 256))
    l2a = chunked(l2a_full)
    l2b = chunked(l2b_full)
    fc2bias = fc2_b.reshape(128, 1).astype(np.float32)

    l3 = np.zeros((128, 16), np.float32)
    l3[:, 0:10] = fc3_w.T                  # [u4, 10]
    l3a = l3.astype(bf)
    l3b = (l3 - l3a.astype(np.float32)).astype(bf)
    fc3bias = np.zeros((16, 1), np.float32)
    fc3bias[0:10, 0] = fc3_b

    common = dict(wc1=wc1, b1s=b1s, t1=t1, wc2=wc2, b2s=b2s, t2=t2,
                  fc1w=fc1wt, fc1b=fc1b, fc2a=l2a, fc2b=l2b, fc2bias=fc2bias,
                  fc3a=l3a, fc3b=l3b, fc3bias=fc3bias)

    # x: pad to [BL, 66, 34] bf16 per core
    xs = x.reshape(B, H, W)
    in_maps = []
    for c in range(N_CORES):
        xc = xs[c * BL:(c + 1) * BL]
        xp = np.zeros((BL + 1, HP, WP), np.float32)
        xp[:BL, 1:H + 1, 1:W + 1] = xc
        m = dict(common)
        m["xp"] = xp.reshape(BL + 1, HP * WP).astype(bf)
        in_maps.append(m)
    return in_maps


_NC_CACHE = {}


def _get_nc():
    if "nc" not in _NC_CACHE:
        _NC_CACHE["nc"] = build_program()
    return _NC_CACHE["nc"]


def kernel(**inputs):
    nc = _get_nc()
    in_maps = _prep_inputs(**inputs)
    res = run_bass_kernel_spmd(nc, in_maps, core_ids=list(range(N_CORES)))
    outs = []
    for c in range(N_CORES):
        o = res.results[c]["out"]            # [16, NS*BL]
        o = o.reshape(16, NS, BL)[0:10]      # [10, NS, BL]
        outs.append(o.transpose(1, 2, 0))    # [NS, BL, 10]
    return np.concatenate(outs, axis=1).astype(np.float32)  # [NS, B, 10]


# revision 17
# speedup vs baseline: 1.4351x; 1.1179x over previous
"""AudioSNN forward pass on 8 Trainium2 NeuronCores (pure data parallel).

Host side: weight re-layout + padding (numpy). Device side: Bass/Tile kernel
per core over a 128-batch shard.
"""
import os
import sys
import numpy as np

for _p in ("/opt/trn_rl_repo", "/root/.axon_site/_ro/trn_rl_repo"):
    if os.path.isdir(_p) and _p not in sys.path:
        sys.path.insert(0, _p)

import ml_dtypes
from contextlib import ExitStack

import concourse.bass as bass
import concourse.tile as tile
from concourse import mybir, bacc
from concourse.bass_utils import run_bass_kernel_spmd

BF16 = mybir.dt.bfloat16
F32 = mybir.dt.float32
GT = mybir.AluOpType.is_gt
ADD = mybir.AluOpType.add
SUB = mybir.AluOpType.subtract
MUL = mybir.AluOpType.mult
SIGMOID = mybir.ActivationFunctionType.Sigmoid

N_CORES = 8
B = 1024
BL = B // N_CORES        # 128 batch per core
H, W = 64, 32            # conv1 image
HP, WP = H + 3, W + 3    # padded (67, 35); conv1 uses a 4x4 tap window
H2, W2 = 32, 16          # conv2 image (after pool1)
H2P, W2P = H2 + 2, W2 + 2  # 34, 18
H3, W3 = 16, 8           # after pool2
NS = 25
BETA = 0.95
SCALE = float(2.0 ** 96)  # sigmoid saturation scale (exact power of two)

BC = 32                  # conv2 batch-chunk
NCHUNK = BL // BC        # 4
P1COLS = H2P * W2P * BC  # 34*18*32 = 19584
GROUPS = BL // 4         # 32 conv1 groups (4 batches each)

# spike engine split: tiles with index % MOD == 0 go to ACT, rest to DVE
ACT_MOD1 = 2
ACT_MOD2 = 2


def build_program():
    nc = bacc.Bacc()

    # ---- inputs (host-preprocessed layouts) ----
    XP = nc.declare_dram_parameter("xp", [BL + 1, HP * WP], BF16, isOutput=False)
    WC1 = nc.declare_dram_parameter("wc1", [128, 128], BF16, isOutput=False)
    B1S = nc.declare_dram_parameter("b1s", [128, 1], F32, isOutput=False)   # -SCALE*thr1
    T1 = nc.declare_dram_parameter("t1", [128, 1], F32, isOutput=False)     # thr1
    WC2 = nc.declare_dram_parameter("wc2", [128, 576], BF16, isOutput=False)  # 4 reps x [32ch, 9 taps x 64]
    B2S = nc.declare_dram_parameter("b2s", [128, 1], F32, isOutput=False)   # -SCALE*thr2
    T2 = nc.declare_dram_parameter("t2", [128, 1], F32, isOutput=False)     # thr2
    FC1W = nc.declare_dram_parameter("fc1w", [2 * 64, 128 * 128], F32, isOutput=False)
    FC1B = nc.declare_dram_parameter("fc1b", [128, 2], F32, isOutput=False)
    FC2A = nc.declare_dram_parameter("fc2a", [128, 256], BF16, isOutput=False)  # lhsT hi [u3_low, (chunk, u4)]
    FC2B = nc.declare_dram_parameter("fc2b", [128, 256], BF16, isOutput=False)  # lo
    FC2BIAS = nc.declare_dram_parameter("fc2bias", [128, 1], F32, isOutput=False)
    FC3A = nc.declare_dram_parameter("fc3a", [128, 16], BF16, isOutput=False)   # lhsT hi [u4, 10->16]
    FC3B = nc.declare_dram_parameter("fc3b", [128, 16], BF16, isOutput=False)
    FC3BIAS = nc.declare_dram_parameter("fc3bias", [16, 1], F32, isOutput=False)

    OUT = nc.declare_dram_parameter("out", [16, NS * BL], F32, isOutput=True)

    dbg = os.environ.get("KERNEL_DEBUG", "")
    dbg_outs = {}
    if dbg:
        dbg_outs["p1"] = nc.declare_dram_parameter("dbg_p1", [128, P1COLS], BF16, isOutput=True)
        dbg_outs["c2"] = nc.declare_dram_parameter("dbg_c2", [128, 64 * BL], F32, isOutput=True)
        dbg_outs["cur3"] = nc.declare_dram_parameter("dbg_cur3", [256, BL], F32, isOutput=True)

    with tile.TileContext(nc) as tc, ExitStack() as top:
        consts = top.enter_context(tc.tile_pool(name="consts", bufs=1))

        wc1 = consts.tile([128, 128], BF16, tag="wc1")
        nc.sync.dma_start(wc1[:], WC1[:])
        b1s = consts.tile([128, 1], F32, tag="b1s")
        nc.sync.dma_start(b1s[:], B1S[:])
        t1 = consts.tile([128, 1], F32, tag="t1")
        nc.sync.dma_start(t1[:], T1[:])
        wc2 = consts.tile([128, 576], BF16, tag="wc2")
        nc.sync.dma_start(wc2[:], WC2[:])
        b2s = consts.tile([128, 1], F32, tag="b2s")
        nc.sync.dma_start(b2s[:], B2S[:])
        t2 = consts.tile([128, 1], F32, tag="t2")
        nc.sync.dma_start(t2[:], T2[:])

        # c2 accumulation buffer: [128 = 2x64ch, 64 rounds x 128 batch] fp32
        c2buf = top.enter_context(tc.tile_pool(name="c2bufp", bufs=1)).tile(
            [128, 64 * BL], F32, tag="c2buf")

        # ------------- conv1 + spike1 + pool1 + conv2 + spike2 + pool2 -------------
        with ExitStack() as convs:
            p1pool = convs.enter_context(tc.tile_pool(name="p1pool", bufs=1))
            xrep_pool = convs.enter_context(tc.tile_pool(name="xrep", bufs=3))
            s1pool = convs.enter_context(tc.tile_pool(name="s1pool", bufs=3))
            pwpool = convs.enter_context(tc.tile_pool(name="pwpool", bufs=2))
            stgpool = convs.enter_context(tc.tile_pool(name="stgpool", bufs=1))
            c1ps = convs.enter_context(tc.tile_pool(name="c1ps", bufs=2, space="PSUM"))
            c2ps = convs.enter_context(tc.tile_pool(name="c2ps", bufs=4, space="PSUM"))
            s2pool = convs.enter_context(tc.tile_pool(name="s2pool", bufs=4))
            pw2pool = convs.enter_context(tc.tile_pool(name="pw2pool", bufs=4))

            # persistent double-buffered P1 + padded pool1 staging; pads are
            # zeroed once and never overwritten afterwards.
            p1bufs = [p1pool.tile([128, P1COLS], BF16, tag=f"p1_{i}", name=f"p1_{i}")
                      for i in range(2)]
            for i in range(2):
                nc.gpsimd.memset(p1bufs[i][0:32, :], 0.0)
            stages = [stgpool.tile([128, H2 * W2P], BF16, tag=f"stg_{i}", name=f"stg_{i}")
                      for i in range(3)]
            for i in range(3):
                nc.vector.memset(stages[i][:], 0.0)

            for chunk in range(NCHUNK):
                # P1: [128 = 4 replicas x 32ch, (b, h2p, w2p)] bf16, pool-sums 0..4
                p1 = p1bufs[chunk % 2]
                p1v = p1[:].rearrange("p (b h w) -> p b h w", b=BC, h=H2P, w=W2P)

                for gg in range(BC // 8):
                    # xr rows: 64*sub + 4*k + b; each row holds a contiguous
                    # 64x35 window of padded x starting at (dy, dx)
                    XW = H * WP  # 2240
                    xr = xrep_pool.tile([128, XW], BF16, tag="xr")
                    xrv = xr[:].rearrange("p (h w) -> p h w", h=H, w=WP)
                    xsrc = XP[:]
                    for sub_ in range(2):
                        for k in range(16):
                            dy, dx = k // 4, k % 4
                            src = bass.AP(xsrc.tensor,
                                          xsrc.offset + (chunk * BC + gg * 8 + 4 * sub_) * (HP * WP) + dy * WP + dx,
                                          [[HP * WP, 4], [1, XW]])
                            r0 = 64 * sub_ + 4 * k
                            nc.sync.dma_start(xr[r0:r0 + 4, :], src)

                    for sub in range(2):
                        sb = 64 * sub
                        # conv1: 4 matmuls N=512 into 2 psum tiles [128, 1024]
                        s1 = s1pool.tile([128, H * W], BF16, tag="s1")
                        for half in range(2):
                            ps = c1ps.tile([128, 1024], F32, tag="c1ps")
                            for q in range(2):
                                h0 = 16 * (2 * half + q)
                                nc.tensor.matmul(ps[:, q * 512:(q + 1) * 512],
                                                 wc1[sb:sb + 64, :],
                                                 xrv[sb:sb + 64, h0:h0 + 16, 0:W],
                                                 start=True, stop=True,
                                                 tile_position=(sb, 0))
                            # spike1 -> {0,1} bf16
                            tgt = s1[:, half * 1024:(half + 1) * 1024]
                            nc.scalar.activation(tgt, ps[:], SIGMOID, bias=b1s[:], scale=SCALE)

                        # pool1 w-pairs: [128, 64h, 16w']
                        pw = pwpool.tile([128, H * (W // 2)], BF16, tag="pw")
                        s1v = s1[:].rearrange("p (h w2 t) -> p h w2 t", h=H, w2=W // 2, t=2)
                        pwv = pw[:].rearrange("p (h w2) -> p h w2", h=H)
                        nc.vector.tensor_add(pwv, s1v[:, :, :, 0], s1v[:, :, :, 1])
                        # pool1 h-pairs into padded staging [128, (32h, 18w)]
                        stg = stages[(gg * 2 + sub) % 3]
                        sv = stg[:].rearrange("p (h2 v) -> p h2 v", h2=H2, v=W2P)
                        pwh = pw[:].rearrange("p (h2 t w2) -> p h2 t w2", h2=H // 2, t=2, w2=W // 2)
                        nc.vector.tensor_add(sv[:, :, 1:W2 + 1], pwh[:, :, 0, :], pwh[:, :, 1, :])

                        # scatter into P1 replica 0 (interior rows), per batch
                        for b4 in range(4):
                            bb = gg * 8 + sub * 4 + b4
                            dst = bass.AP(p1.tensor, p1.offset + bb * (H2P * W2P) + W2P,
                                          [[P1COLS, 32], [1, H2 * W2P]])
                            nc.sync.dma_start(dst, stg[32 * b4:32 * (b4 + 1), :])

                # replicate P1 block 0 -> blocks 1..3
                for rep in range(1, 4):
                    nc.sync.dma_start(p1[32 * rep:32 * (rep + 1), :], p1[0:32, :])
                if dbg and chunk == 0:
                    nc.sync.dma_start(dbg_outs["p1"][:], p1[:])

                # ---- conv2 on this chunk: 4 h-rows interleaved across row groups ----
                c2q = c2buf[:].rearrange("p (j4 q b) -> p j4 q b", q=4, b=BL)
                for quad in range(H2 // 4):
                    ps2s = [c2ps.tile([64, W2 * BC], F32, tag="c2ps",
                                      name=f"ps2_{chunk}_{quad}_{r}") for r in range(4)]
                    for k in range(9):
                        dy, dx = k // 3, k % 3
                        for r in range(4):
                            hrow = 4 * quad + r
                            base = 32 * r
                            w_ap = wc2[base:base + 32, 64 * k:64 * (k + 1)]
                            rhs = p1v[base:base + 32, :, hrow + dy, dx:dx + W2]
                            nc.tensor.matmul(ps2s[r][:], w_ap, rhs,
                                             start=(k == 0), stop=(k == 8),
                                             tile_position=(base, 0))
                    pw2s = []
                    for r in range(4):
                        hrow = 4 * quad + r
                        # spike2 -> {0,1} bf16
                        s2 = s2pool.tile([64, W2 * BC], BF16, tag="s2")
                        nc.scalar.activation(s2[:], ps2s[r][:], SIGMOID,
                                             bias=b2s[0:64, :], scale=SCALE)
                        # pool2 w-pairs -> [64, (BC, 8w')]
                        pw2 = pw2pool.tile([64, (W2 // 2) * BC], BF16, tag="pw2",
                                           name=f"pw2_{chunk}_{hrow}")
                        s2v = s2[:].rearrange("p (b w2 t) -> p b w2 t", b=BC, w2=W2 // 2, t=2)
                        pw2v = pw2[:].rearrange("p (b w2) -> p b w2", b=BC)
                        nc.vector.tensor_add(pw2v, s2v[:, :, :, 0], s2v[:, :, :, 1])
                        pw2s.append(pw2)
                    # pool2 h-pairs -> c2buf strided, fp32
                    # even w' -> partitions 0:64, odd w' -> partitions 64:128
                    for half2 in range(2):
                        j = 2 * quad + half2
                        pa = pw2s[2 * half2][:].rearrange("p (b w4 t) -> p b w4 t", b=BC, w4=W2 // 4, t=2)
                        pb = pw2s[2 * half2 + 1][:].rearrange("p (b w4 t) -> p b w4 t", b=BC, w4=W2 // 4, t=2)
                        for par in range(2):
                            dst = c2q[64 * par:64 * (par + 1), j,
                                      :, chunk * BC:(chunk + 1) * BC].squeeze()
                            dst = dst.transpose([0, 2, 1])
                            nc.vector.tensor_add(dst, pa[:, :, :, par], pb[:, :, :, par])

        if dbg:
            nc.sync.dma_start(dbg_outs["c2"][:], c2buf[:])

        # ---------------- fc1 (fp32) + LIF ----------------
        with ExitStack() as fcs:
            fc1wpool = fcs.enter_context(tc.tile_pool(name="fc1w", bufs=6))
            fc1ps = fcs.enter_context(tc.tile_pool(name="fc1ps", bufs=1, space="PSUM"))
            lifps = fcs.enter_context(tc.tile_pool(name="lifps", bufs=2, space="PSUM"))
            lifc = fcs.enter_context(tc.tile_pool(name="lifc", bufs=1))

            fc1b = consts.tile([128, 2], F32, tag="fc1b")
            nc.sync.dma_start(fc1b[:], FC1B[:])

            cur3 = [lifc.tile([128, BL], F32, tag=f"cur3_{h}", name=f"cur3_{h}") for h in range(2)]
            for h in range(2):
                pA = fc1ps.tile([128, BL], F32, tag="fc1psA")
                pB = fc1ps.tile([128, BL], F32, tag="fc1psB")
                c2r = c2buf[:].rearrange("p (r b) -> p r b", b=BL)
                for r in range(64):
                    wt = fc1wpool.tile([128, 128], F32, tag="fc1wt")
                    nc.sync.dma_start(wt[:], FC1W[64 * h + r, :].rearrange("(p m) -> p m", p=128))
                    nc.tensor.matmul(pA[:], wt[0:64, :], c2r[0:64, r, :],
                                     start=(r == 0), stop=(r == 63), tile_position=(0, 0))
                    nc.tensor.matmul(pB[:], wt[64:128, :], c2r[64:128, r, :],
                                     start=(r == 0), stop=(r == 63), tile_position=(64, 0))
                tsum = lifc.tile([128, BL], F32, tag=f"tsum_{h}", name=f"tsum_{h}")
                # cur3 = 0.25*(pA + pB) + fc1_b, one PSUM input per op
                nc.vector.tensor_scalar(tsum[:], pA[:], 0.25, fc1b[:, h:h + 1],
                                        op0=MUL, op1=ADD)
                nc.vector.scalar_tensor_tensor(cur3[h][:], pB[:], 0.25, tsum[:],
                                               op0=MUL, op1=ADD)
            if dbg:
                nc.sync.dma_start(dbg_outs["cur3"][0:128, :], cur3[0][:])
                nc.sync.dma_start(dbg_outs["cur3"][128:256, :], cur3[1][:])

            # LIF state + weights
            fc2a = consts.tile([128, 256], BF16, tag="fc2a")
            nc.sync.dma_start(fc2a[:], FC2A[:])
            fc2b_w = consts.tile([128, 256], BF16, tag="fc2b_w")
            nc.sync.dma_start(fc2b_w[:], FC2B[:])
            fc2bias = consts.tile([128, 1], F32, tag="fc2bias")
            nc.sync.dma_start(fc2bias[:], FC2BIAS[:])
            fc3a = consts.tile([128, 16], BF16, tag="fc3a")
            nc.sync.dma_start(fc3a[:], FC3A[:])
            fc3b_w = consts.tile([128, 16], BF16, tag="fc3b_w")
            nc.sync.dma_start(fc3b_w[:], FC3B[:])
            fc3bias = consts.tile([16, 1], F32, tag="fc3bias")
            nc.sync.dma_start(fc3bias[:], FC3BIAS[:])

            mem3 = [lifc.tile([128, BL], F32, tag=f"mem3_{h}", name=f"mem3_{h}") for h in range(2)]
            spk3 = [lifc.tile([128, BL], BF16, tag=f"spk3_{h}", name=f"spk3_{h}") for h in range(2)]
            mem4 = lifc.tile([128, BL], F32, tag="mem4")
            spk4 = lifc.tile([128, BL], BF16, tag="spk4")
            mem5 = lifc.tile([16, BL], F32, tag="mem5")
            spk5 = lifc.tile([16, BL], F32, tag="spk5")
            outstage = lifc.tile([16, NS * BL], F32, tag="outstage")
            for t_ in mem3 + spk3 + [mem4, spk4, mem5, spk5]:
                nc.vector.memset(t_[:], 0.0)
            nc.vector.memset(outstage[:], 0.0)

            t3 = [lifc.tile([128, BL], F32, tag=f"t3_{h}", name=f"t3_{h}") for h in range(2)]
            t4 = lifc.tile([128, BL], F32, tag="t4")
            t5 = lifc.tile([16, BL], F32, tag="t5")

            for step in range(NS):
                # layer 3 (cur3 constant across steps)
                for h in range(2):
                    nc.vector.scalar_tensor_tensor(t3[h][:], mem3[h][:], BETA, cur3[h][:],
                                                   op0=MUL, op1=ADD)
                    nc.vector.tensor_sub(mem3[h][:], t3[h][:], spk3[h][:])
                    nc.vector.tensor_scalar(spk3[h][:], mem3[h][:], 1.0, None, op0=GT)
                # fc2 (hi/lo split)
                p4 = lifps.tile([128, BL], F32, tag="p4")
                nc.tensor.matmul(p4[:], fc2a[:, 0:128], spk3[0][:], start=True, stop=False)
                nc.tensor.matmul(p4[:], fc2a[:, 128:256], spk3[1][:], start=False, stop=False)
                nc.tensor.matmul(p4[:], fc2b_w[:, 0:128], spk3[0][:], start=False, stop=False)
                nc.tensor.matmul(p4[:], fc2b_w[:, 128:256], spk3[1][:], start=False, stop=True)
                # layer 4
                nc.vector.tensor_scalar(t4[:], mem4[:], BETA, fc2bias[:], op0=MUL, op1=ADD)
                nc.vector.tensor_add(t4[:], t4[:], p4[:])
                nc.vector.tensor_sub(mem4[:], t4[:], spk4[:])
                nc.vector.tensor_scalar(spk4[:], mem4[:], 1.0, None, op0=GT)
                # fc3
                p5 = lifps.tile([16, BL], F32, tag="p5")
                nc.tensor.matmul(p5[:], fc3a[:], spk4[:], start=True, stop=False)
                nc.tensor.matmul(p5[:], fc3b_w[:], spk4[:], start=False, stop=True)
                # layer 5
                nc.vector.tensor_scalar(t5[:], mem5[:], BETA, fc3bias[:], op0=MUL, op1=ADD)
                nc.vector.tensor_add(t5[:], t5[:], p5[:])
                nc.vector.tensor_sub(mem5[:], t5[:], spk5[:])
                nc.vector.tensor_scalar(spk5[:], mem5[:], 1.0, None, op0=GT)
                nc.vector.tensor_copy(outstage[:, step * BL:(step + 1) * BL], spk5[:])

            nc.sync.dma_start(OUT[:], outstage[:])

    nc.compile()
    return nc


def _prep_inputs(x, conv1_w, conv1_b, conv2_w, conv2_b, fc1_w, fc1_b,
                 fc2_w, fc2_b, fc3_w, fc3_b):
    """Host-side preprocessing -> list of 8 per-core input dicts."""
    bf = ml_dtypes.bfloat16

    # conv1 weights: [128, 128]: 2 replicas of block-diag [64 = 4b x 16taps, 128]
    wc1 = np.zeros((128, 128), np.float32)
    w1 = conv1_w.reshape(32, 3, 3)  # [c, dy, dx]
    for sub in range(2):
        for dy in range(3):
            for dx in range(3):
                k = 4 * dy + dx
                for b4 in range(4):
                    wc1[64 * sub + 4 * k + b4, 32 * b4:32 * (b4 + 1)] = w1[:, dy, dx]
    wc1 = wc1.astype(bf)

    thr1 = (1.0 - conv1_b).astype(np.float32)          # [32]
    t1 = np.tile(thr1, 4).reshape(128, 1).astype(np.float32)
    b1s = (-(t1.astype(np.float64)) * SCALE).astype(np.float32)

    # conv2 weights: [128 = 4 reps x 32ch_in, 9 taps x 64 ch_out]
    wtap = conv2_w.transpose(1, 2, 3, 0).reshape(32, 9, 64)  # [c_in, k, c_out]
    wc2_one = wtap.reshape(32, 576)
    wc2 = np.tile(wc2_one, (4, 1)).astype(bf)
    thr2 = (4.0 * (1.0 - conv2_b)).astype(np.float32)  # [64]
    t2 = np.tile(thr2, 2).reshape(128, 1).astype(np.float32)
    b2s = (-(t2.astype(np.float64)) * SCALE).astype(np.float32)

    # fc1 weights: [2*64 rounds, 128 part * 128 units] fp32
    # part p: ch = p%64, hw' = 2r + p//64; feat = ch*128 + hw'; unit = 128h + m
    fw = fc1_w.reshape(256, 64, 128)       # [u, ch, hw']
    fc1wt = np.zeros((2, 64, 128, 128), np.float32)
    for h in range(2):
        us = fw[128 * h:128 * (h + 1)]     # [128u, 64ch, 128hw]
        for r in range(64):
            fc1wt[h, r, 0:64, :] = us[:, :, 2 * r].T       # [ch, u]
            fc1wt[h, r, 64:128, :] = us[:, :, 2 * r + 1].T
    fc1wt = fc1wt.reshape(2 * 64, 128 * 128)
    fc1b = np.ascontiguousarray(fc1_b.reshape(2, 128).T).astype(np.float32)

    # fc2: lhsT [u3, u4]; hi/lo split
    l2 = np.ascontiguousarray(fc2_w.T).astype(np.float32)   # [256 u3, 128 u4]
    l2a_full = l2.astype(bf)
    l2b_full = (l2 - l2a_full.astype(np.float32)).astype(bf)
    def chunked(a):  # [256, 128] -> [128, 256] with chunk-major cols
        return np.ascontiguousarray(a.reshape(2, 128, 128).transpose(1, 0, 2).reshape(128, 256))
    l2a = chunked(l2a_full)
    l2b = chunked(l2b_full)
    fc2bias = fc2_b.reshape(128, 1).astype(np.float32)

    l3 = np.zeros((128, 16), np.float32)
    l3[:, 0:10] = fc3_w.T                  # [u4, 10]
    l3a = l3.astype(bf)
    l3b = (l3 - l3a.astype(np.float32)).astype(bf)
    fc3bias = np.zeros((16, 1), np.float32)
    fc3bias[0:10, 0] = fc3_b

    common = dict(wc1=wc1, b1s=b1s, t1=t1, wc2=wc2, b2s=b2s, t2=t2,
                  fc1w=fc1wt, fc1b=fc1b, fc2a=l2a, fc2b=l2b, fc2bias=fc2bias,
                  fc3a=l3a, fc3b=l3b, fc3bias=fc3bias)

    # x: pad to [BL, 66, 34] bf16 per core
    xs = x.reshape(B, H, W)
    in_maps = []
    for c in range(N_CORES):
        xc = xs[c * BL:(c + 1) * BL]
        xp = np.zeros((BL + 1, HP, WP), np.float32)
        xp[:BL, 1:H + 1, 1:W + 1] = xc
        m = dict(common)
        m["xp"] = xp.reshape(BL + 1, HP * WP).astype(bf)
        in_maps.append(m)
    return in_maps


_NC_CACHE = {}


def _get_nc():
    if "nc" not in _NC_CACHE:
        _NC_CACHE["nc"] = build_program()
    return _NC_CACHE["nc"]


def kernel(**inputs):
    nc = _get_nc()
    in_maps = _prep_inputs(**inputs)
    res = run_bass_kernel_spmd(nc, in_maps, core_ids=list(range(N_CORES)))
    outs = []
    for c in range(N_CORES):
        o = res.results[c]["out"]            # [16, NS*BL]
        o = o.reshape(16, NS, BL)[0:10]      # [10, NS, BL]
        outs.append(o.transpose(1, 2, 0))    # [NS, BL, 10]
    return np.concatenate(outs, axis=1).astype(np.float32)  # [NS, B, 10]
